# revision 33
# baseline (speedup 1.0000x reference)
"""CRPS loss kernel for Trainium2, data-parallel over 8 NeuronCores.

Math (per sample n, timestep t, quantiles q_0..q_10 sorted, target y):
  trapz(n,t) = 0.5 * sum_i q_i * (beta'_i + w_i) - min(q_0, y) + max(y, q_10)
  where O_i = [q_i >= y], u_i = G_i*O_i, G_i = 1 - i/5,
        w_0 = -(u_0+u_1), w_i = u_{i-1} - u_{i+1} (1<=i<=9), w_10 = u_9+u_10,
        beta' = [1.99, -0.04, -0.08, ..., -0.36, -0.19]
  out[n] = mean_t trapz(n,t)

Layout: each core gets 512 samples; 4 blocks of 128 samples; partition =
sample, free = (t=512, i=11) with per-sample data contiguous in DRAM.
All reductions via tensor_tensor_reduce accum_out -> [128,1] per block.
"""
import sys

if "/opt/trn_rl_repo" not in sys.path:
    sys.path.insert(0, "/opt/trn_rl_repo")

import numpy as np
import concourse.bass as bass
import concourse.tile as tile
from concourse import bacc, mybir
from concourse.bass_utils import run_bass_kernel_spmd
from concourse.alu_op_type import AluOpType

N_CORES = 8
N, T, D = 4096, 512, 11
N_LOC = N // N_CORES        # 512 samples per core
P = 128                     # partitions
BLOCKS = N_LOC // P         # 4
FP = mybir.dt.float32
BF = mybir.dt.bfloat16

# quantile-axis constants (see module docstring), scaled by LAM=100 so all
# values are small integers — exactly representable in bf16.  u, w and w'
# stay integer-valued <= 256 in magnitude, so the bf16 pipeline is exact.
LAM = 100.0
_F = np.arange(D) / 10.0
_A = _F * _F
_G = 1.0 - 2.0 * _F
_BETA = np.empty(D)
_BETA[0] = -(_A[0] + _A[1])
for _j in range(1, D - 1):
    _BETA[_j] = _A[_j - 1] - _A[_j + 1]
_BETA[D - 1] = _A[D - 2] + _A[D - 1] - 2.0
_GT = np.rint(LAM * _G)      # [100, 80, ..., -100]
_BT = np.rint(LAM * _BETA)   # [199, -4, -8, ..., -36, -19]

N_DMA_SPLIT = 8  # parallel DMA queues for the big Q load
DMA_MODE = "free"  # "free": split along free dim; "part": along partitions
PARTS = "full"   # debug knob: "full" | "nostt" | "dma"


def _pair(flat_ap, a, b):
    """[P, 2, T] view of a [P, T*D] tile selecting quantile columns {a, b}."""
    part = flat_ap.ap[0]
    return bass.AP(flat_ap.tensor, flat_ap.offset + a, [part, [b - a, 2], [D, T]])


def build_consts(tc, const_pool):
    """Materialize full-width bf16 constant tiles (one-time)."""
    nc = tc.nc
    g_s = const_pool.tile([P, D], BF, tag="gs")
    b_s = const_pool.tile([P, D], BF, tag="bs")
    for i in range(D):
        nc.vector.memset(g_s[:, i : i + 1], float(_GT[i]))
        nc.vector.memset(b_s[:, i : i + 1], float(_BT[i]))
    g_f = const_pool.tile([P, T * D], BF, tag="gf")
    b_f = const_pool.tile([P, T * D], BF, tag="bf")
    g_f3 = g_f[:].rearrange("p (t i) -> p t i", i=D)
    b_f3 = b_f[:].rearrange("p (t i) -> p t i", i=D)
    nc.vector.tensor_copy(g_f3, g_s[:].unsqueeze(1).broadcast_to([P, T, D]))
    nc.vector.tensor_copy(b_f3, b_s[:].unsqueeze(1).broadcast_to([P, T, D]))
    return g_f, b_f


def build_crps_kernel(tc, out_ap, inp_ap, tgt_ap, g_f, b_f):
    nc = tc.nc

    inp_r = inp_ap.rearrange("(b p) t i -> b p (t i)", p=P)   # [4, 128, 5632]
    tgt_r = tgt_ap.rearrange("(b p) t -> b p t", p=P)          # [4, 128, 512]
    out_r = out_ap.rearrange("(b p) -> b p", p=P)              # [4, 128]

    with (
        tc.tile_pool(name="data", bufs=3) as data_pool,
        tc.tile_pool(name="work", bufs=2) as work_pool,
        tc.tile_pool(name="acc", bufs=2) as acc_pool,
    ):
        # per-block accumulators land in one [P, BLOCKS] tile each; the
        # final combine + output DMA run once per iteration, not per block
        s_w4 = acc_pool.tile([P, BLOCKS], FP, tag="sw4")
        s_min4 = acc_pool.tile([P, BLOCKS], FP, tag="smin4")
        s_max4 = acc_pool.tile([P, BLOCKS], FP, tag="smax4")
        for b in range(BLOCKS):
            q = data_pool.tile([P, T * D], FP, tag="q")
            if DMA_MODE == "part":
                # split along partitions: each DMA is one fully-linear
                # contiguous DRAM read of (128/N)*22528 bytes
                pg = P // N_DMA_SPLIT
                for s in range(N_DMA_SPLIT):
                    nc.sync.dma_start(
                        q[s * pg : (s + 1) * pg, :],
                        inp_r[b][s * pg : (s + 1) * pg, :],
                    )
            else:
                chunk = (T * D) // N_DMA_SPLIT
                for s in range(N_DMA_SPLIT):
                    nc.sync.dma_start(
                        q[:, s * chunk : (s + 1) * chunk],
                        inp_r[b][:, s * chunk : (s + 1) * chunk],
                    )
            if PARTS == "dmaq":
                if b == 0:
                    r3 = acc_pool.tile([P, 1], FP, tag="r3")
                    nc.vector.tensor_copy(r3[:], q[:, 0:1])
                    nc.sync.dma_start(out_r[b].unsqueeze(1), r3[:])
                continue

            tg = data_pool.tile([P, T], FP, tag="tg")
            nc.sync.dma_start(tg[:], tgt_r[b])

            q3 = q[:].rearrange("p (t i) -> p t i", i=D)
            tgB = tg[:].unsqueeze(2).broadcast_to([P, T, D])

            if PARTS == "dma":
                r3 = acc_pool.tile([P, 1], FP, tag="r3")
                nc.vector.tensor_copy(r3[:], q3[:, 0:1, 0:1].squeeze(2))
                nc.sync.dma_start(out_r[b].unsqueeze(1), r3[:])
                continue

            # O = [q >= y]  (exact f32 compare, bf16 0/1 output)
            o = work_pool.tile([P, T * D], BF, tag="o")
            o3 = o[:].rearrange("p (t i) -> p t i", i=D)
            nc.vector.tensor_tensor(o3, q3, tgB, AluOpType.is_ge)

            # u = Gt * O   (bf16 2x, exact: integers <= 100)
            u = work_pool.tile([P, T * D], BF, tag="u")
            u3 = u[:].rearrange("p (t i) -> p t i", i=D)
            nc.vector.tensor_tensor(u[:], o[:], g_f[:], AluOpType.mult)

            # w built in place of o (o is dead after u); integer diffs <= 200
            # boundary columns use transposed [1, 512] APs: a [512, 1] AP
            # pays ~4x per-row overhead on DVE, the long-inner form doesn't
            w3 = o3
            uT = u[:].rearrange("p (t i) -> p i t", i=D)
            wT = o[:].rearrange("p (t i) -> p i t", i=D)
            nc.vector.tensor_tensor(
                w3[:, :, 1:10], u3[:, :, 0:9], u3[:, :, 2:11], AluOpType.subtract
            )
            # boundary columns with the relu(q0-t)/relu(t-q10) q-parts folded
            # in (sign flips vs the plain trapz form):
            # w'_0 = u_0 - u_1 ; w'_10 = u_9 - u_10
            nc.vector.scalar_tensor_tensor(
                wT[:, 0:1, :], uT[:, 0:1, :], 1.0, uT[:, 1:2, :],
                AluOpType.mult, AluOpType.subtract,
            )
            nc.vector.tensor_tensor(
                wT[:, 10:11, :], uT[:, 9:10, :], uT[:, 10:11, :],
                AluOpType.subtract,
            )
            # t-branch of the relu folds: TB = sum_t t - sum_t t*(O_0+O_10)
            # d = u_0 - u_10 = 100*(O_0 + O_10), contiguous f32 out
            dmm = acc_pool.tile([P, T], FP, tag="dmm")
            nc.vector.tensor_tensor(
                dmm[:].unsqueeze(1), uT[:, 0:1, :], uT[:, 10:11, :],
                AluOpType.subtract,
            )

            # w' = w + beta_t  (bf16 2x, exact: |w'| <= 236) -> into u tile
            wp = u
            nc.vector.tensor_tensor(wp[:], o[:], b_f[:], AluOpType.add)
            wp3 = u3

            if PARTS == "nostt":
                r3 = acc_pool.tile([P, 1], FP, tag="r3")
                nc.vector.tensor_copy(r3[:], wp3[:, 0:1, 0:1].squeeze(2))
                nc.sync.dma_start(out_r[b].unsqueeze(1), r3[:])
                continue

            # S = sum_{t,i} q * w'   (f32 x bf16 fused multiply+accumulate;
            # scr is a dead scratch output, bf16 to save SBUF)
            scr = work_pool.tile([P, T * D], BF, tag="scr")
            scr3 = scr[:].rearrange("p (t i) -> p t i", i=D)
            nc.vector.scalar_tensor_tensor(
                scr3, q3, 1.0, wp3, AluOpType.mult, AluOpType.mult,
                accum_out=s_w4[:, b : b + 1],
            )
            # ST1 = sum_t t*100*(O_0+O_10)  (contiguous f32 x f32 stt)
            scr2 = scr[:, : 2 * T]
            nc.vector.scalar_tensor_tensor(
                scr2[:, 0:T], tg[:], 1.0, dmm[:],
                AluOpType.mult, AluOpType.mult, accum_out=s_min4[:, b : b + 1],
            )
            # ST0 = sum_t t  (single-src with accumulate)
            nc.vector.tensor_scalar(
                scr2[:, T : 2 * T], tg[:], 1.0, 0.0, AluOpType.mult,
                AluOpType.add, accum_out=s_max4[:, b : b + 1],
            )

        # r = (S/(2*LAM) + ST0 - ST1/100) / T for all blocks at once
        r1 = acc_pool.tile([P, BLOCKS], FP, tag="r1")
        nc.vector.scalar_tensor_tensor(
            r1[:], s_min4[:], -1.0 / LAM, s_max4[:],
            AluOpType.mult, AluOpType.add,
        )
        r2 = acc_pool.tile([P, BLOCKS], FP, tag="r2")
        nc.vector.scalar_tensor_tensor(
            r2[:], s_w4[:], 0.5 / LAM, r1[:], AluOpType.mult, AluOpType.add,
        )
        r3 = acc_pool.tile([P, BLOCKS], FP, tag="r3")
        nc.vector.tensor_scalar_mul(r3[:], r2[:], 1.0 / T)
        # out[(b p)]: partition p writes BLOCKS floats at stride P
        nc.sync.dma_start(out_ap.rearrange("(b p) -> p b", p=P), r3[:])


def _build_nc(repeat=1):
    nc = bacc.Bacc("TRN2", target_bir_lowering=False, debug=False,
                   num_devices=N_CORES)
    inp = nc.dram_tensor("inp", [N_LOC, T, D], FP, kind="ExternalInput").ap()
    tgt = nc.dram_tensor("target", [N_LOC, T], FP, kind="ExternalInput").ap()
    out = nc.dram_tensor("out", [N_LOC], FP, kind="ExternalOutput").ap()
    with tile.TileContext(nc) as tc:
        with tc.tile_pool(name="const", bufs=1) as const_pool:
            g_f, b_f = build_consts(tc, const_pool)
            if repeat == 1:
                build_crps_kernel(tc, out, inp, tgt, g_f, b_f)
            else:
                with tc.For_i(0, repeat, 1):
                    build_crps_kernel(tc, out, inp, tgt, g_f, b_f)
    nc.compile()
    return nc


_NC_CACHE = {}


def get_nc(repeat=1):
    if repeat not in _NC_CACHE:
        _NC_CACHE[repeat] = _build_nc(repeat)
    return _NC_CACHE[repeat]


def kernel(inp: np.ndarray, target: np.ndarray) -> np.ndarray:
    inp = np.ascontiguousarray(inp, dtype=np.float32)
    target = np.ascontiguousarray(target, dtype=np.float32)
    nc = get_nc()
    in_maps = [
        {
            "inp": inp[c * N_LOC : (c + 1) * N_LOC],
            "target": target[c * N_LOC : (c + 1) * N_LOC],
        }
        for c in range(N_CORES)
    ]
    res = run_bass_kernel_spmd(nc, in_maps, core_ids=list(range(N_CORES)))
    return np.concatenate([res.results[c]["out"] for c in range(N_CORES)])


# revision 34
# speedup vs baseline: 1.3000x; 1.3000x over previous
"""CRPS loss kernel for Trainium2, data-parallel over 8 NeuronCores.

Math (per sample n, timestep t, quantiles q_0..q_10 sorted, target y):
  trapz(n,t) = 0.5 * sum_i q_i * (beta'_i + w_i) - min(q_0, y) + max(y, q_10)
  where O_i = [q_i >= y], u_i = G_i*O_i, G_i = 1 - i/5,
        w_0 = -(u_0+u_1), w_i = u_{i-1} - u_{i+1} (1<=i<=9), w_10 = u_9+u_10,
        beta' = [1.99, -0.04, -0.08, ..., -0.36, -0.19]
  out[n] = mean_t trapz(n,t)

Layout: each core gets 512 samples; 4 blocks of 128 samples; partition =
sample, free = (t=512, i=11) with per-sample data contiguous in DRAM.
All reductions via tensor_tensor_reduce accum_out -> [128,1] per block.
"""
import sys

if "/opt/trn_rl_repo" not in sys.path:
    sys.path.insert(0, "/opt/trn_rl_repo")

import numpy as np
import concourse.bass as bass
import concourse.tile as tile
from concourse import bacc, mybir
from concourse.bass_utils import run_bass_kernel_spmd
from concourse.alu_op_type import AluOpType

N_CORES = 8
N, T, D = 4096, 512, 11
N_LOC = N // N_CORES        # 512 samples per core
P = 128                     # partitions
BLOCKS = N_LOC // P         # 4
FP = mybir.dt.float32
BF = mybir.dt.bfloat16

# quantile-axis constants (see module docstring), scaled by LAM=100 so all
# values are small integers — exactly representable in bf16.  u, w and w'
# stay integer-valued <= 256 in magnitude, so the bf16 pipeline is exact.
LAM = 100.0
_F = np.arange(D) / 10.0
_A = _F * _F
_G = 1.0 - 2.0 * _F
_BETA = np.empty(D)
_BETA[0] = -(_A[0] + _A[1])
for _j in range(1, D - 1):
    _BETA[_j] = _A[_j - 1] - _A[_j + 1]
_BETA[D - 1] = _A[D - 2] + _A[D - 1] - 2.0
_GT = np.rint(LAM * _G)      # [100, 80, ..., -100]
_BT = np.rint(LAM * _BETA)   # [199, -4, -8, ..., -36, -19]

N_DMA_SPLIT = 8  # parallel DMA queues for the big Q load
DMA_MODE = "free"  # "free": split along free dim; "part": along partitions
PARTS = "full"   # debug knob: "full" | "nostt" | "dma"


def _pair(flat_ap, a, b):
    """[P, 2, T] view of a [P, T*D] tile selecting quantile columns {a, b}."""
    part = flat_ap.ap[0]
    return bass.AP(flat_ap.tensor, flat_ap.offset + a, [part, [b - a, 2], [D, T]])


def build_consts(tc, const_pool):
    """Materialize full-width bf16 constant tiles (one-time)."""
    nc = tc.nc
    g_s = const_pool.tile([P, D], BF, tag="gs")
    b_s = const_pool.tile([P, D], BF, tag="bs")
    for i in range(D):
        nc.vector.memset(g_s[:, i : i + 1], float(_GT[i]))
        nc.vector.memset(b_s[:, i : i + 1], float(_BT[i]))
    g_f = const_pool.tile([P, T * D], BF, tag="gf")
    b_f = const_pool.tile([P, T * D], BF, tag="bf")
    g_f3 = g_f[:].rearrange("p (t i) -> p t i", i=D)
    b_f3 = b_f[:].rearrange("p (t i) -> p t i", i=D)
    nc.vector.tensor_copy(g_f3, g_s[:].unsqueeze(1).broadcast_to([P, T, D]))
    nc.vector.tensor_copy(b_f3, b_s[:].unsqueeze(1).broadcast_to([P, T, D]))
    return g_f, b_f


def build_crps_kernel(tc, out_ap, inp_ap, tgt_ap, g_f, b_f):
    nc = tc.nc

    inp_r = inp_ap.rearrange("(b p) t i -> b p (t i)", p=P)   # [4, 128, 5632]
    tgt_r = tgt_ap.rearrange("(b p) t -> b p t", p=P)          # [4, 128, 512]
    out_r = out_ap.rearrange("(b p) -> b p", p=P)              # [4, 128]

    with (
        tc.tile_pool(name="data", bufs=3) as data_pool,
        tc.tile_pool(name="work", bufs=2) as work_pool,
        tc.tile_pool(name="acc", bufs=2) as acc_pool,
    ):
        # per-block accumulators land in one [P, BLOCKS] tile each; the
        # final combine + output DMA run once per iteration, not per block
        s_w4 = acc_pool.tile([P, BLOCKS], FP, tag="sw4")
        s_min4 = acc_pool.tile([P, BLOCKS], FP, tag="smin4")
        s_max4 = acc_pool.tile([P, BLOCKS], FP, tag="smax4")
        for b in range(BLOCKS):
            q = data_pool.tile([P, T * D], FP, tag="q")
            if DMA_MODE == "part":
                # split along partitions: each DMA is one fully-linear
                # contiguous DRAM read of (128/N)*22528 bytes
                pg = P // N_DMA_SPLIT
                for s in range(N_DMA_SPLIT):
                    nc.sync.dma_start(
                        q[s * pg : (s + 1) * pg, :],
                        inp_r[b][s * pg : (s + 1) * pg, :],
                    )
            else:
                chunk = (T * D) // N_DMA_SPLIT
                for s in range(N_DMA_SPLIT):
                    nc.sync.dma_start(
                        q[:, s * chunk : (s + 1) * chunk],
                        inp_r[b][:, s * chunk : (s + 1) * chunk],
                    )
            if PARTS == "dmaq":
                if b == 0:
                    r3 = acc_pool.tile([P, 1], FP, tag="r3")
                    nc.vector.tensor_copy(r3[:], q[:, 0:1])
                    nc.sync.dma_start(out_r[b].unsqueeze(1), r3[:])
                continue

            tg = data_pool.tile([P, T], FP, tag="tg")
            nc.sync.dma_start(tg[:], tgt_r[b])

            q3 = q[:].rearrange("p (t i) -> p t i", i=D)
            tgB = tg[:].unsqueeze(2).broadcast_to([P, T, D])

            if PARTS == "dma":
                r3 = acc_pool.tile([P, 1], FP, tag="r3")
                nc.vector.tensor_copy(r3[:], q3[:, 0:1, 0:1].squeeze(2))
                nc.sync.dma_start(out_r[b].unsqueeze(1), r3[:])
                continue

            # O = [q >= y]  (exact f32 compare, bf16 0/1 output)
            o = work_pool.tile([P, T * D], BF, tag="o")
            o3 = o[:].rearrange("p (t i) -> p t i", i=D)
            nc.vector.tensor_tensor(o3, q3, tgB, AluOpType.is_ge)

            # u = Gt * O   (bf16 2x, exact: integers <= 100)
            u = work_pool.tile([P, T * D], BF, tag="u")
            u3 = u[:].rearrange("p (t i) -> p t i", i=D)
            nc.vector.tensor_tensor(u[:], o[:], g_f[:], AluOpType.mult)

            # w built in place of o (o is dead after u); integer diffs <= 200
            # boundary columns use transposed [1, 512] APs: a [512, 1] AP
            # pays ~4x per-row overhead on DVE, the long-inner form doesn't
            w3 = o3
            uT = u[:].rearrange("p (t i) -> p i t", i=D)
            wT = o[:].rearrange("p (t i) -> p i t", i=D)
            nc.vector.tensor_tensor(
                w3[:, :, 1:10], u3[:, :, 0:9], u3[:, :, 2:11], AluOpType.subtract
            )
            # boundary columns with the relu(q0-t)/relu(t-q10) q-parts folded
            # in (sign flips vs the plain trapz form):
            # w'_0 = u_0 - u_1 ; w'_10 = u_9 - u_10
            nc.vector.scalar_tensor_tensor(
                wT[:, 0:1, :], uT[:, 0:1, :], 1.0, uT[:, 1:2, :],
                AluOpType.mult, AluOpType.subtract,
            )
            nc.vector.tensor_tensor(
                wT[:, 10:11, :], uT[:, 9:10, :], uT[:, 10:11, :],
                AluOpType.subtract,
            )
            # t-branch of the relu folds: TB = sum_t t - sum_t t*(O_0+O_10)
            # d = u_0 - u_10 = 100*(O_0 + O_10), contiguous f32 out
            dmm = acc_pool.tile([P, T], FP, tag="dmm")
            nc.vector.tensor_tensor(
                dmm[:].unsqueeze(1), uT[:, 0:1, :], uT[:, 10:11, :],
                AluOpType.subtract,
            )

            # w' = w + beta_t  (bf16 2x, exact: |w'| <= 236) -> into u tile
            wp = u
            nc.vector.tensor_tensor(wp[:], o[:], b_f[:], AluOpType.add)
            wp3 = u3

            if PARTS == "nostt":
                r3 = acc_pool.tile([P, 1], FP, tag="r3")
                nc.vector.tensor_copy(r3[:], wp3[:, 0:1, 0:1].squeeze(2))
                nc.sync.dma_start(out_r[b].unsqueeze(1), r3[:])
                continue

            # S = sum_{t,i} q * w'   (f32 x bf16 fused multiply+accumulate;
            # scr is a dead scratch output, bf16 to save SBUF)
            scr = work_pool.tile([P, T * D], BF, tag="scr")
            scr3 = scr[:].rearrange("p (t i) -> p t i", i=D)
            nc.vector.scalar_tensor_tensor(
                scr3, q3, 1.0, wp3, AluOpType.mult, AluOpType.mult,
                accum_out=s_w4[:, b : b + 1],
            )
            # ST1 = sum_t t*100*(O_0+O_10)  (contiguous f32 x f32 stt)
            scr2 = scr[:, : 2 * T]
            nc.vector.scalar_tensor_tensor(
                scr2[:, 0:T], tg[:], 1.0, dmm[:],
                AluOpType.mult, AluOpType.mult, accum_out=s_min4[:, b : b + 1],
            )
            # ST0 = sum_t t  (single-src with accumulate)
            nc.vector.tensor_scalar(
                scr2[:, T : 2 * T], tg[:], 1.0, 0.0, AluOpType.mult,
                AluOpType.add, accum_out=s_max4[:, b : b + 1],
            )

        # r = (S/(2*LAM) + ST0 - ST1/100) / T for all blocks at once
        r1 = acc_pool.tile([P, BLOCKS], FP, tag="r1")
        nc.vector.scalar_tensor_tensor(
            r1[:], s_min4[:], -1.0 / LAM, s_max4[:],
            AluOpType.mult, AluOpType.add,
        )
        r2 = acc_pool.tile([P, BLOCKS], FP, tag="r2")
        nc.vector.scalar_tensor_tensor(
            r2[:], s_w4[:], 0.5 / LAM, r1[:], AluOpType.mult, AluOpType.add,
        )
        r3 = acc_pool.tile([P, BLOCKS], FP, tag="r3")
        nc.vector.tensor_scalar_mul(r3[:], r2[:], 1.0 / T)
        # out[(b p)]: partition p writes BLOCKS floats at stride P
        nc.sync.dma_start(out_ap.rearrange("(b p) -> p b", p=P), r3[:])


def _build_nc(repeat=1):
    nc = bacc.Bacc("TRN2", target_bir_lowering=False, debug=False,
                   num_devices=N_CORES)
    inp = nc.dram_tensor("inp", [N_LOC, T, D], FP, kind="ExternalInput").ap()
    tgt = nc.dram_tensor("target", [N_LOC, T], FP, kind="ExternalInput").ap()
    out = nc.dram_tensor("out", [N_LOC], FP, kind="ExternalOutput").ap()
    with tile.TileContext(nc) as tc:
        with tc.tile_pool(name="const", bufs=1) as const_pool:
            g_f, b_f = build_consts(tc, const_pool)
            if repeat == 1:
                build_crps_kernel(tc, out, inp, tgt, g_f, b_f)
            else:
                with tc.For_i(0, repeat, 1, staggered_reset=True):
                    build_crps_kernel(tc, out, inp, tgt, g_f, b_f)
    nc.compile()
    return nc


_NC_CACHE = {}


def get_nc(repeat=1):
    if repeat not in _NC_CACHE:
        _NC_CACHE[repeat] = _build_nc(repeat)
    return _NC_CACHE[repeat]


def kernel(inp: np.ndarray, target: np.ndarray) -> np.ndarray:
    inp = np.ascontiguousarray(inp, dtype=np.float32)
    target = np.ascontiguousarray(target, dtype=np.float32)
    nc = get_nc()
    in_maps = [
        {
            "inp": inp[c * N_LOC : (c + 1) * N_LOC],
            "target": target[c * N_LOC : (c + 1) * N_LOC],
        }
        for c in range(N_CORES)
    ]
    res = run_bass_kernel_spmd(nc, in_maps, core_ids=list(range(N_CORES)))
    return np.concatenate([res.results[c]["out"] for c in range(N_CORES)])
